# revision 3
# baseline (speedup 1.0000x reference)
"""BatchMultiHeadGraphAttention Trainium2 kernel.

Data-parallel over batch: 8 batches -> 8 NeuronCores, one batch per core.

Per core (batch b), per (i-tile t of 128 rows, head h):
  scores   x[i,j] = s_i + d_j           (PE, K=2 outer-product matmul)
           where s = h @ (w[h] @ a_src[h]), d = h @ (w[h] @ a_dst[h])
           (folded on host into sdw [64,16]; s/d rows computed on-device)
  u        = Relation * x               (DVE tensor_tensor, psum x)
  m        = leaky_relu(u, 0.2)         (ACT Prelu alpha=0.2; exact since R>=0:
                                         R*leaky(x) == leaky(R*x))
  e, den   = exp(m), rowsum             (ACT Exp with accum_out)
  attn32   = e * (1/den)                (GpSimd tensor_scalar; -> DRAM attn)
  attn16   = e * (1/den) in fp16        (DVE; feeds AV matmul path)
  eT       = transpose(attn16) blocks   (PE transpose, fp16, psum->sbuf copies
                                         split DVE/ACT)
  out      = attn16^T.T @ hp16          (PE fp16 matmuls accumulating in psum;
                                         DMA psum -> DRAM directly)
bias_p is added on host (zeros in the reference setup, but handled anyway).
"""

import os
import sys

import numpy as np

for _p in ("/opt/trn_rl_repo", "/root/.axon_site/_ro/trn_rl_repo"):
    if os.path.isdir(_p) and _p not in sys.path:
        sys.path.insert(0, _p)

import concourse.bass as bass  # noqa: E402
import concourse.tile as tile  # noqa: E402
from concourse import bacc, mybir  # noqa: E402
from concourse.bass_utils import run_bass_kernel_spmd  # noqa: E402
from concourse.masks import make_identity  # noqa: E402

BS, N, H, F = 8, 1024, 8, 64
P = 128
NT = N // P  # 8 row tiles of 128
F32 = mybir.dt.float32
F16 = mybir.dt.float16

_CACHE = {}


def _build():
    nc = bacc.Bacc("TRN2", target_bir_lowering=False, debug=False)
    hT_d = nc.dram_tensor("hT", [F, N], F32, kind="ExternalInput").ap()
    R_d = nc.dram_tensor("R", [N, N], F32, kind="ExternalInput").ap()
    w_d = nc.dram_tensor("w", [F, H * F], F32, kind="ExternalInput").ap()
    sdw_d = nc.dram_tensor("sdw", [F, 2 * H], F32, kind="ExternalInput").ap()
    attn_d = nc.dram_tensor("attn", [H, N, N], F32, kind="ExternalOutput").ap()
    out_d = nc.dram_tensor("out", [H, N, F], F32, kind="ExternalOutput").ap()

    with tile.TileContext(nc) as tc:
        with (
            tc.tile_pool(name="singles", bufs=1) as singles,
            tc.tile_pool(name="work", bufs=2) as work,
            tc.tile_pool(name="small", bufs=3) as small,
            tc.tile_pool(name="ps_x", bufs=2, space="PSUM") as ps_x,
            tc.tile_pool(name="ps_t", bufs=2, space="PSUM") as ps_t,
            tc.tile_pool(name="ps_av", bufs=2, space="PSUM") as ps_av,
        ):
            # ---------- setup ----------
            hT_sb = singles.tile([F, N], F32, tag="hT")
            w_sb = singles.tile([F, H * F], F32, tag="w")
            sdw_sb = singles.tile([F, 2 * H], F32, tag="sdw")
            nc.sync.dma_start(out=hT_sb, in_=hT_d)
            nc.sync.dma_start(out=w_sb, in_=w_d)
            nc.sync.dma_start(out=sdw_sb, in_=sdw_d)

            ident = singles.tile([P, P], F16, tag="ident")
            make_identity(nc, ident)

            # s/d row vectors for all heads: [16, N]; rows 0-7 = s_h, 8-15 = d_h
            ps_sd = ps_x.tile([16, N], F32, tag="x")
            for half in range(2):
                sl = slice(half * 512, (half + 1) * 512)
                nc.tensor.matmul(
                    ps_sd[:, sl], sdw_sb[:], hT_sb[:, sl], start=True, stop=True
                )
            sd_sb = singles.tile([16, N], F32, tag="sd")
            nc.vector.tensor_copy(sd_sb[:], ps_sd[:])

            # h_prime in fp16, per j-tile: hp16[nt][:, 64h:64h+64] = (h @ w[h])[j-tile]
            hp16 = []
            for nt in range(NT):
                ps_hp = ps_x.tile([P, H * F], F32, tag="x")
                for h in range(H):
                    nc.tensor.matmul(
                        ps_hp[:, h * F : (h + 1) * F],
                        hT_sb[:, nt * P : (nt + 1) * P],
                        w_sb[:, h * F : (h + 1) * F],
                        start=(h == 0),
                        stop=(h == H - 1),
                    )
                t16 = singles.tile([P, H * F], F16, tag=f"hp{nt}")
                nc.vector.tensor_copy(t16[:], ps_hp[:])
                hp16.append(t16)

            # score matmul operands per head:
            #   stage_l[h] [2, N]: row0 = s_h, row1 = ones   (lhsT source)
            #   stage_r[h] [2, N]: row0 = ones, row1 = d_h   (rhs source)
            stage_l, stage_r = [], []
            for h in range(H):
                sl_t = singles.tile([2, N], F32, tag=f"sl{h}")
                sr_t = singles.tile([2, N], F32, tag=f"sr{h}")
                nc.gpsimd.memset(sl_t[:], 1.0)
                nc.gpsimd.memset(sr_t[:], 1.0)
                nc.sync.dma_start(out=sl_t[0:1, :], in_=sd_sb[h : h + 1, :])
                nc.sync.dma_start(out=sr_t[1:2, :], in_=sd_sb[H + h : H + h + 1, :])
                stage_l.append(sl_t)
                stage_r.append(sr_t)

            # ---------- main loop ----------
            for t in range(NT):
                R_t = work.tile([P, N], F32, tag="R")
                nc.sync.dma_start(out=R_t, in_=R_d[t * P : (t + 1) * P, :])
                for h in range(H):
                    px = ps_x.tile([P, N], F32, tag="x")
                    for half in range(2):
                        sl = slice(half * 512, (half + 1) * 512)
                        nc.tensor.matmul(
                            px[:, sl],
                            stage_l[h][:, t * P : (t + 1) * P],
                            stage_r[h][:, sl],
                            start=True,
                            stop=True,
                        )
                    u = work.tile([P, N], F32, tag="u")
                    nc.vector.tensor_tensor(
                        out=u[:], in0=px[:], in1=R_t[:], op=mybir.AluOpType.mult
                    )
                    m = work.tile([P, N], F32, tag="m")
                    nc.scalar.activation(
                        m[:], u[:], mybir.ActivationFunctionType.Prelu, alpha=0.2
                    )
                    e = work.tile([P, N], F32, tag="e")
                    den = small.tile([P, 1], F32, tag="den")
                    nc.scalar.activation(
                        e[:], m[:], mybir.ActivationFunctionType.Exp,
                        accum_out=den[:],
                    )
                    rden = small.tile([P, 1], F32, tag="rden")
                    nc.vector.reciprocal(rden[:], den[:])

                    attn32 = work.tile([P, N], F32, tag="a32")
                    nc.gpsimd.tensor_scalar_mul(attn32[:], e[:], rden[:])
                    nc.sync.dma_start(
                        out=attn_d[h, t * P : (t + 1) * P, :], in_=attn32[:]
                    )

                    attn16 = work.tile([P, N], F16, tag="a16")
                    nc.vector.tensor_scalar_mul(attn16[:], e[:], rden[:])

                    eT = work.tile([P, N], F16, tag="eT")
                    for jb in range(NT):
                        pt = ps_t.tile([P, P], F16, tag="t")
                        nc.tensor.transpose(
                            pt[:], attn16[:, jb * P : (jb + 1) * P], ident[:]
                        )
                        dst = eT[:, jb * P : (jb + 1) * P]
                        if jb % 2 == 0:
                            nc.vector.tensor_copy(dst, pt[:])
                        else:
                            nc.scalar.copy(dst, pt[:])

                    pav = ps_av.tile([P, F], F32, tag="av")
                    for jb in range(NT):
                        nc.tensor.matmul(
                            pav[:],
                            eT[:, jb * P : (jb + 1) * P],
                            hp16[jb][:, h * F : (h + 1) * F],
                            start=(jb == 0),
                            stop=(jb == NT - 1),
                        )
                    o_sb = small.tile([P, F], F32, tag="o")
                    if h % 2 == 0:
                        nc.vector.tensor_copy(o_sb[:], pav[:])
                    else:
                        nc.scalar.copy(o_sb[:], pav[:])
                    nc.sync.dma_start(
                        out=out_d[h, t * P : (t + 1) * P, :], in_=o_sb[:]
                    )
    nc.compile()
    return nc


def _get_nc():
    if "nc" not in _CACHE:
        _CACHE["nc"] = _build()
    return _CACHE["nc"]


def kernel(h, Relation, w, a_src, a_dst, bias_p, _trace=False):
    h = np.asarray(h, dtype=np.float32)
    Relation = np.ascontiguousarray(np.asarray(Relation, dtype=np.float32))
    w = np.asarray(w, dtype=np.float32)
    a_src = np.asarray(a_src, dtype=np.float32)
    a_dst = np.asarray(a_dst, dtype=np.float32)
    bias_p = np.asarray(bias_p, dtype=np.float32)

    hT = np.ascontiguousarray(h.transpose(0, 2, 1))  # [BS, F, N]
    ws = np.einsum("hfo,ho->hf", w, a_src[..., 0])  # [H, F]
    wd = np.einsum("hfo,ho->hf", w, a_dst[..., 0])
    sdw = np.ascontiguousarray(
        np.concatenate([ws.T, wd.T], axis=1), dtype=np.float32
    )  # [F, 16]
    w_pack = np.ascontiguousarray(
        w.transpose(1, 0, 2).reshape(F, H * F), dtype=np.float32
    )

    nc = _get_nc()
    in_maps = [
        {"hT": hT[b], "R": Relation[b], "w": w_pack, "sdw": sdw} for b in range(BS)
    ]
    res = run_bass_kernel_spmd(nc, in_maps, core_ids=list(range(BS)))

    output = np.stack([res.results[b]["out"] for b in range(BS)])  # [BS,H,N,F]
    attn = np.stack([res.results[b]["attn"] for b in range(BS)])  # [BS,H,N,N]
    output = output + bias_p[None, None, None, :]
    return output, attn


# revision 5
# speedup vs baseline: 1.3965x; 1.3965x over previous
"""BatchMultiHeadGraphAttention Trainium2 kernel.

Data-parallel over batch: 8 batches -> 8 NeuronCores, one batch per core.

Per core (batch b), per (i-tile t of 128 rows, head h):
  scores   x[i,j] = s_i + d_j       (PE, K=2 outer-product matmuls in float32r
                                     with hi+lo residual compensation -> exact
                                     fp32 scores at 1 cycle/column)
           where s = h @ (w[h] @ a_src[h]), d = h @ (w[h] @ a_dst[h])
           (folded on host into sdw [64,16]; s/d rows computed on-device)
  u        = Relation * x           (DVE tensor_tensor, psum x)
  m        = leaky_relu(u, 0.2)     (ACT Prelu alpha=0.2; exact since R>=0:
                                     R*leaky(x) == leaky(R*x))
  e, den   = exp(m), rowsum         (ACT Exp with accum_out)
  attn32   = e * (1/den)            (GpSimd tensor_scalar; -> DRAM attn)
  attn16   = e * (1/den) in fp16    (DVE; feeds AV matmul path)
  eT       = transpose(attn16)      (PE transpose, 8 blocks into one fp16 psum
                                     bank; one batched psum->sbuf copy per tile
                                     alternating DVE/ACT)
  out      = eT.T @ hp16            (PE fp16 matmuls accumulating in psum)
bias_p is added on host (zeros in the reference setup, but handled anyway).
"""

import os
import sys

import numpy as np

for _p in ("/opt/trn_rl_repo", "/root/.axon_site/_ro/trn_rl_repo"):
    if os.path.isdir(_p) and _p not in sys.path:
        sys.path.insert(0, _p)

import concourse.bass as bass  # noqa: E402
import concourse.tile as tile  # noqa: E402
from concourse import bacc, mybir  # noqa: E402
from concourse.bass_utils import run_bass_kernel_spmd  # noqa: E402
from concourse.masks import make_identity  # noqa: E402

BS, N, H, F = 8, 1024, 8, 64
P = 128
NT = N // P  # 8 row tiles of 128
F32 = mybir.dt.float32
F32R = mybir.dt.float32r
F16 = mybir.dt.float16

_CACHE = {}


def _build():
    nc = bacc.Bacc("TRN2", target_bir_lowering=False, debug=False)
    hT_d = nc.dram_tensor("hT", [F, N], F32, kind="ExternalInput").ap()
    R_d = nc.dram_tensor("R", [N, N], F32, kind="ExternalInput").ap()
    w_d = nc.dram_tensor("w", [F, H * F], F32, kind="ExternalInput").ap()
    sdw_d = nc.dram_tensor("sdw", [F, 2 * H], F32, kind="ExternalInput").ap()
    ones_d = nc.dram_tensor("ones", [1, N], F32R, kind="ExternalInput").ap()
    attn_d = nc.dram_tensor("attn", [H, N, N], F32, kind="ExternalOutput").ap()
    out_d = nc.dram_tensor("out", [H, N, F], F32, kind="ExternalOutput").ap()

    with tile.TileContext(nc) as tc:
        with (
            tc.tile_pool(name="singles", bufs=1) as singles,
            tc.tile_pool(name="work", bufs=2) as work,
            tc.tile_pool(name="stage", bufs=2) as stage,
            tc.tile_pool(name="small", bufs=3) as small,
            tc.tile_pool(name="ps_x", bufs=2, space="PSUM") as ps_x,
            tc.tile_pool(name="ps_t", bufs=2, space="PSUM") as ps_t,
            tc.tile_pool(name="ps_av", bufs=2, space="PSUM") as ps_av,
        ):
            # ---------- setup ----------
            hT_sb = singles.tile([F, N], F32, tag="hT")
            w_sb = singles.tile([F, H * F], F32, tag="w")
            sdw_sb = singles.tile([F, 2 * H], F32, tag="sdw")
            ones_r = singles.tile([1, N], F32R, tag="ones")
            nc.sync.dma_start(out=hT_sb, in_=hT_d)
            nc.sync.dma_start(out=w_sb, in_=w_d)
            nc.sync.dma_start(out=sdw_sb, in_=sdw_d)
            nc.sync.dma_start(out=ones_r, in_=ones_d)

            ident = singles.tile([P, P], F16, tag="ident")
            make_identity(nc, ident)

            # s/d row vectors for all heads: [16, N]; rows 0-7 = s_h, 8-15 = d_h
            ps_sd = ps_x.tile([16, N], F32, tag="x")
            for half in range(2):
                sl = slice(half * 512, (half + 1) * 512)
                nc.tensor.matmul(
                    ps_sd[:, sl], sdw_sb[:], hT_sb[:, sl], start=True, stop=True
                )
            sd_sb = singles.tile([16, N], F32, tag="sd")
            nc.vector.tensor_copy(sd_sb[:], ps_sd[:])
            # f32r rounding + residual for exact-score compensation
            sd_hi = singles.tile([16, N], F32R, tag="sdhi")
            nc.vector.tensor_copy(sd_hi[:], sd_sb[:])
            sd_lo32 = singles.tile([16, N], F32, tag="sdlo32")
            nc.vector.tensor_tensor(
                out=sd_lo32[:],
                in0=sd_sb[:],
                in1=sd_hi[:].bitcast(F32),
                op=mybir.AluOpType.subtract,
            )
            sd_lo = singles.tile([16, N], F32R, tag="sdlo")
            nc.vector.tensor_copy(sd_lo[:], sd_lo32[:])

            # h_prime in fp16, per j-tile: hp16[nt][:, 64h:64h+64] = (h @ w[h])[j-tile]
            hp16 = []
            for nt in range(NT):
                ps_hp = ps_x.tile([P, H * F], F32, tag="x")
                for h in range(H):
                    nc.tensor.matmul(
                        ps_hp[:, h * F : (h + 1) * F],
                        hT_sb[:, nt * P : (nt + 1) * P],
                        w_sb[:, h * F : (h + 1) * F],
                        start=(h == 0),
                        stop=(h == H - 1),
                    )
                t16 = singles.tile([P, H * F], F16, tag=f"hp{nt}")
                nc.vector.tensor_copy(t16[:], ps_hp[:])
                hp16.append(t16)

            # all Relation row-tiles resident (8 x 4KB/partition)
            R_all = []
            for t in range(NT):
                R_t = singles.tile([P, N], F32, tag=f"R{t}")
                nc.sync.dma_start(out=R_t, in_=R_d[t * P : (t + 1) * P, :])
                R_all.append(R_t)

            # ---------- main loop (h outer so score operands stage once/head) --
            it = 0
            for h in range(H):
                # score operands for head h (f32r), hi and lo:
                #   l_* [2, N]: row0 = s-part, row1 = ones   (lhsT source)
                #   r_* [2, N]: row0 = ones, row1 = d-part   (rhs source)
                stg = {}
                for kind, src_row, data_row in (
                    ("lhi", sd_hi[h : h + 1, :], 0),
                    ("rhi", sd_hi[H + h : H + h + 1, :], 1),
                    ("llo", sd_lo[h : h + 1, :], 0),
                    ("rlo", sd_lo[H + h : H + h + 1, :], 1),
                ):
                    st = stage.tile([2, N], F32R, tag=kind)
                    nc.sync.dma_start(
                        out=st[data_row : data_row + 1, :], in_=src_row
                    )
                    nc.sync.dma_start(
                        out=st[1 - data_row : 2 - data_row, :], in_=ones_r[:]
                    )
                    stg[kind] = st
                for t in range(NT):
                    R_t = R_all[t]
                    tsl = slice(t * P, (t + 1) * P)
                    px = ps_x.tile([P, N], F32, tag="x")
                    for half in range(2):
                        sl = slice(half * 512, (half + 1) * 512)
                        nc.tensor.matmul(
                            px[:, sl], stg["lhi"][:, tsl], stg["rhi"][:, sl],
                            start=True, stop=False,
                        )
                        nc.tensor.matmul(
                            px[:, sl], stg["llo"][:, tsl], stg["rlo"][:, sl],
                            start=False, stop=True,
                        )
                    u = work.tile([P, N], F32, tag="u")
                    nc.vector.tensor_tensor(
                        out=u[:], in0=px[:], in1=R_t[:], op=mybir.AluOpType.mult
                    )
                    m = work.tile([P, N], F32, tag="m")
                    nc.scalar.activation(
                        m[:], u[:], mybir.ActivationFunctionType.Prelu, alpha=0.2
                    )
                    e = work.tile([P, N], F32, tag="e")
                    den = small.tile([P, 1], F32, tag="den")
                    nc.scalar.activation(
                        e[:], m[:], mybir.ActivationFunctionType.Exp,
                        accum_out=den[:],
                    )
                    rden = small.tile([P, 1], F32, tag="rden")
                    nc.vector.reciprocal(rden[:], den[:])

                    attn32 = work.tile([P, N], F32, tag="a32")
                    nc.gpsimd.tensor_scalar_mul(attn32[:], e[:], rden[:])
                    nc.sync.dma_start(out=attn_d[h, tsl, :], in_=attn32[:])

                    attn16 = work.tile([P, N], F16, tag="a16")
                    nc.vector.tensor_scalar_mul(attn16[:], e[:], rden[:])

                    # 8 transposed blocks into ONE fp16 psum bank
                    pt = ps_t.tile([P, N], F16, tag="t")
                    for jb in range(NT):
                        nc.tensor.matmul(
                            pt[:, jb * P : (jb + 1) * P],
                            attn16[:, jb * P : (jb + 1) * P],
                            ident[:],
                            is_transpose=True,
                            start=(jb == 0),
                            stop=(jb == NT - 1),
                        )
                    eT = work.tile([P, N], F16, tag="eT")
                    if it % 2 == 0:
                        nc.vector.tensor_copy(eT[:], pt[:])
                    else:
                        nc.scalar.copy(eT[:], pt[:])

                    pav = ps_av.tile([P, F], F32, tag="av")
                    for jb in range(NT):
                        nc.tensor.matmul(
                            pav[:],
                            eT[:, jb * P : (jb + 1) * P],
                            hp16[jb][:, h * F : (h + 1) * F],
                            start=(jb == 0),
                            stop=(jb == NT - 1),
                        )
                    o_sb = small.tile([P, F], F32, tag="o")
                    if it % 2 == 0:
                        nc.scalar.copy(o_sb[:], pav[:])
                    else:
                        nc.vector.tensor_copy(o_sb[:], pav[:])
                    nc.sync.dma_start(out=out_d[h, tsl, :], in_=o_sb[:])
                    it += 1
    nc.compile()
    return nc


def _get_nc():
    if "nc" not in _CACHE:
        _CACHE["nc"] = _build()
    return _CACHE["nc"]


def kernel(h, Relation, w, a_src, a_dst, bias_p):
    h = np.asarray(h, dtype=np.float32)
    Relation = np.ascontiguousarray(np.asarray(Relation, dtype=np.float32))
    w = np.asarray(w, dtype=np.float32)
    a_src = np.asarray(a_src, dtype=np.float32)
    a_dst = np.asarray(a_dst, dtype=np.float32)
    bias_p = np.asarray(bias_p, dtype=np.float32)

    hT = np.ascontiguousarray(h.transpose(0, 2, 1))  # [BS, F, N]
    ws = np.einsum("hfo,ho->hf", w, a_src[..., 0])  # [H, F]
    wd = np.einsum("hfo,ho->hf", w, a_dst[..., 0])
    sdw = np.ascontiguousarray(
        np.concatenate([ws.T, wd.T], axis=1), dtype=np.float32
    )  # [F, 16]
    w_pack = np.ascontiguousarray(
        w.transpose(1, 0, 2).reshape(F, H * F), dtype=np.float32
    )
    ones = np.ones((1, N), dtype=np.float32)

    nc = _get_nc()
    in_maps = [
        {"hT": hT[b], "R": Relation[b], "w": w_pack, "sdw": sdw, "ones": ones}
        for b in range(BS)
    ]
    res = run_bass_kernel_spmd(nc, in_maps, core_ids=list(range(BS)))

    output = np.stack([res.results[b]["out"] for b in range(BS)])  # [BS,H,N,F]
    attn = np.stack([res.results[b]["attn"] for b in range(BS)])  # [BS,H,N,N]
    output = output + bias_p[None, None, None, :]
    return output, attn


# revision 8
# speedup vs baseline: 1.4089x; 1.0088x over previous
"""BatchMultiHeadGraphAttention Trainium2 kernel.

Data-parallel over batch: 8 batches -> 8 NeuronCores, one batch per core.

Per core (batch b), head h outer, i-tile t (128 rows) inner:
  scores   x[i,j] = s_i + d_j     PE: K=2 outer-product matmuls in float32r,
                                  hi+lo residual compensation -> exact fp32
                                  (fp32 matmuls are ~10x slower on HW; f32r
                                  rounds to ~tf32, the lo pass restores fp32)
           where s = h @ (w[h] @ a_src[h]), d = h @ (w[h] @ a_dst[h])
           (folded on host into sdw [64,16]; s/d rows built on-device)
  m        = leaky_relu(x * R)    DVE custom op MUL_LRELU (== R*leaky(x), R>=0)
  e        = exp(m)               ACT (no accum_out -- it costs ~4us/call on HW)
  e16      = e * 2^-7 in fp16     DVE (range-safe unnormalized; no recip dep)
  eT       = transpose(e16)       PE transpose, 8 blocks into one fp16 psum
                                  bank; one batched psum->sbuf copy (ACT)
  av|den   = eT.T @ [hp16 | 1]    PE fp16 matmuls, 65-wide rhs: the ones
                                  column makes the softmax denominator fall
                                  out of the AV accumulation for free
  rden     = 1/psum[:,64]         DVE (= 2^7/den)
  attn32   = e * rden * 2^-7      DVE -> DRAM attn (paired 1MB DMAs)
  out      = psum[:,0:64] * rden  DVE -> per-head buffer, one DMA per head
bias_p is added on host (zeros in the reference setup, but handled anyway).
GpSimd is avoided for streaming ops (~15x slower than DVE on HW).
"""

import os
import sys

import numpy as np

for _p in ("/opt/trn_rl_repo", "/root/.axon_site/_ro/trn_rl_repo"):
    if os.path.isdir(_p) and _p not in sys.path:
        sys.path.insert(0, _p)

import concourse.bass as bass  # noqa: E402
import concourse.tile as tile  # noqa: E402
from concourse import bacc, mybir  # noqa: E402
from concourse.bass_utils import run_bass_kernel_spmd  # noqa: E402
from concourse.masks import make_identity  # noqa: E402
from concourse import dve_ops as _dve_ops  # noqa: E402
from concourse.dve_spec import Spec, Src0, Src1, C0, maxx, lower as _dve_lower  # noqa: E402
from concourse.dve_uop import DveOpSpec  # noqa: E402

BS, N, H, F = 8, 1024, 8, 64
P = 128
NT = N // P  # 8 row tiles of 128
F32 = mybir.dt.float32
F32R = mybir.dt.float32r
F16 = mybir.dt.float16

_CACHE = {}


def _register_mul_lrelu():
    """Custom DVE op: out = leaky_relu(in0 * in1, s0) = max(u, s0*u), u = in0*in1.

    Fuses the Relation-mask multiply and the leaky relu into one DVE pass.
    """
    name = "MUL_LRELU_ANT"
    for op in _dve_ops.OPS:
        if op.name == name:
            return op
    u = Src0 * Src1
    spec = Spec(
        body=maxx(u, u * C0),
        reference=lambda in0, in1, s0, s1, imm2: np.maximum(
            (in0 * in1), s0 * (in0 * in1)
        ).astype(np.float32),
    )
    op = _dve_ops.DveOp(name, spec, subdim=False, uops_sha={})
    _dve_ops.OPS.append(op)
    _dve_ops._SUB_OPCODE_FOR_NAME[name] = (
        max(_dve_ops._SUB_OPCODE_FOR_NAME.values()) + 1
    )
    _dve_ops.CUSTOM_DVE_SPECS[name] = spec
    for ver in ("v3", "v4"):
        compiled = DveOpSpec(
            name=name,
            opcode=_dve_ops.get_dve_sub_opcode(name),
            uops=_dve_lower(spec, ver=ver),
            rd1_en=True,
        )
        op.uops_sha[ver] = compiled.sha(ver)
    return op


MUL_LRELU = _register_mul_lrelu()


def _build():
    nc = bacc.Bacc("TRN2", target_bir_lowering=False, debug=False)
    hT_d = nc.dram_tensor("hT", [F, N], F32, kind="ExternalInput").ap()
    R_d = nc.dram_tensor("R", [N, N], F32, kind="ExternalInput").ap()
    w_d = nc.dram_tensor("w", [F, H * F], F32, kind="ExternalInput").ap()
    sdw_d = nc.dram_tensor("sdw", [F, 2 * H], F32, kind="ExternalInput").ap()
    ones_d = nc.dram_tensor("ones", [1, N], F32R, kind="ExternalInput").ap()
    attn_d = nc.dram_tensor("attn", [H, N, N], F32, kind="ExternalOutput").ap()
    out_d = nc.dram_tensor("out", [H, N, F], F32, kind="ExternalOutput").ap()

    wb = int(os.environ.get("KB_WORK", "2"))
    with tile.TileContext(nc) as tc:
        with (
            tc.tile_pool(name="singles", bufs=1) as singles,
            tc.tile_pool(name="work", bufs=wb) as work,
            tc.tile_pool(name="stage", bufs=2) as stage,
            tc.tile_pool(name="small", bufs=4) as small,
            tc.tile_pool(name="ps_x", bufs=2, space="PSUM") as ps_x,
            tc.tile_pool(name="ps_t", bufs=2, space="PSUM") as ps_t,
            tc.tile_pool(name="ps_av", bufs=2, space="PSUM") as ps_av,
        ):
            # ---------- setup ----------
            hT_sb = singles.tile([F, N], F32, tag="hT")
            w_sb = singles.tile([F, H * F], F32, tag="w")
            sdw_sb = singles.tile([F, 2 * H], F32, tag="sdw")
            ones_r = singles.tile([1, N], F32R, tag="ones")
            nc.sync.dma_start(out=hT_sb, in_=hT_d)
            nc.sync.dma_start(out=w_sb, in_=w_d)
            nc.sync.dma_start(out=sdw_sb, in_=sdw_d)
            nc.sync.dma_start(out=ones_r, in_=ones_d)

            ident = singles.tile([P, P], F16, tag="ident")
            make_identity(nc, ident)

            # s/d row vectors for all heads: [16, N]; rows 0-7 = s_h, 8-15 = d_h
            ps_sd = ps_x.tile([16, N], F32, tag="x")
            for half in range(2):
                sl = slice(half * 512, (half + 1) * 512)
                nc.tensor.matmul(
                    ps_sd[:, sl], sdw_sb[:], hT_sb[:, sl], start=True, stop=True
                )
            sd_sb = singles.tile([16, N], F32, tag="sd")
            nc.vector.tensor_copy(sd_sb[:], ps_sd[:])
            # f32r rounding + residual for exact-score compensation
            sd_hi = singles.tile([16, N], F32R, tag="sdhi")
            nc.vector.tensor_copy(sd_hi[:], sd_sb[:])
            sd_lo32 = singles.tile([16, N], F32, tag="sdlo32")
            nc.vector.tensor_tensor(
                out=sd_lo32[:],
                in0=sd_sb[:],
                in1=sd_hi[:].bitcast(F32),
                op=mybir.AluOpType.subtract,
            )
            sd_lo = singles.tile([16, N], F32R, tag="sdlo")
            nc.vector.tensor_copy(sd_lo[:], sd_lo32[:])

            # h_prime in fp16 with an interleaved ones column per head:
            # hp16[nt][:, 65h:65h+64] = (h @ w[h])[j-tile], col 65h+64 = 1.0
            hp16 = []
            for nt in range(NT):
                ps_hp = ps_x.tile([P, H * F], F32, tag="x")
                for h in range(H):
                    nc.tensor.matmul(
                        ps_hp[:, h * F : (h + 1) * F],
                        hT_sb[:, nt * P : (nt + 1) * P],
                        w_sb[:, h * F : (h + 1) * F],
                        start=(h == 0),
                        stop=(h == H - 1),
                    )
                t16 = singles.tile([P, H * (F + 1)], F16, tag=f"hp{nt}")
                nc.vector.tensor_copy(
                    t16[:].rearrange("p (h f) -> p h f", f=F + 1)[:, :, 0:F],
                    ps_hp[:].rearrange("p (h f) -> p h f", f=F),
                )
                nc.vector.memset(
                    t16[:].rearrange("p (h f) -> p h f", f=F + 1)[:, :, F : F + 1],
                    1.0,
                )
                hp16.append(t16)

            # all Relation row-tiles resident (8 x 4KB/partition)
            R_all = []
            for t in range(NT):
                R_t = singles.tile([P, N], F32, tag=f"R{t}")
                nc.sync.dma_start(out=R_t, in_=R_d[t * P : (t + 1) * P, :])
                R_all.append(R_t)

            # ---------- main loop (h outer so score operands stage once/head) --
            for h in range(H):
                # score operands for head h (f32r), hi and lo:
                #   l_* [2, N]: row0 = s-part, row1 = ones   (lhsT source)
                #   r_* [2, N]: row0 = ones, row1 = d-part   (rhs source)
                stg = {}
                for kind, src_row, data_row in (
                    ("lhi", sd_hi[h : h + 1, :], 0),
                    ("rhi", sd_hi[H + h : H + h + 1, :], 1),
                    ("llo", sd_lo[h : h + 1, :], 0),
                    ("rlo", sd_lo[H + h : H + h + 1, :], 1),
                ):
                    st = stage.tile([2, N], F32R, tag=kind)
                    nc.sync.dma_start(
                        out=st[data_row : data_row + 1, :], in_=src_row
                    )
                    nc.sync.dma_start(
                        out=st[1 - data_row : 2 - data_row, :], in_=ones_r[:]
                    )
                    stg[kind] = st

                out_h = work.tile([P, NT * F], F32, tag="oh")
                for t in range(NT):
                    R_t = R_all[t]
                    tsl = slice(t * P, (t + 1) * P)

                    px = ps_x.tile([P, N], F32, tag="x")
                    for half in range(2):
                        sl = slice(half * 512, (half + 1) * 512)
                        nc.tensor.matmul(
                            px[:, sl], stg["lhi"][:, tsl], stg["rhi"][:, sl],
                            start=True, stop=False,
                        )
                        nc.tensor.matmul(
                            px[:, sl], stg["llo"][:, tsl], stg["rlo"][:, sl],
                            start=False, stop=True,
                        )

                    m = work.tile([P, N], F32, tag="m")
                    nc.vector._custom_dve(
                        MUL_LRELU, out=m[:], in0=px[:], in1=R_t[:], s0=0.2
                    )
                    e = work.tile([P, N], F32, tag="e")
                    nc.scalar.activation(
                        e[:], m[:], mybir.ActivationFunctionType.Exp
                    )
                    # fp16 copy of e scaled by 2^-7 (range-safe, no recip dep)
                    attn16 = work.tile([P, N], F16, tag="a16")
                    nc.vector.tensor_scalar_mul(attn16[:], e[:], 2.0 ** -7)

                    # 8 transposed blocks into ONE fp16 psum bank
                    pt = ps_t.tile([P, N], F16, tag="t")
                    for jb in range(NT):
                        nc.tensor.matmul(
                            pt[:, jb * P : (jb + 1) * P],
                            attn16[:, jb * P : (jb + 1) * P],
                            ident[:],
                            is_transpose=True,
                            start=(jb == 0),
                            stop=(jb == NT - 1),
                        )
                    eT = work.tile([P, N], F16, tag="eT")
                    nc.scalar.copy(eT[:], pt[:])

                    # AV + denominator: 65-wide rhs, ones column -> den*2^-7
                    pav = ps_av.tile([P, F + 1], F32, tag="av")
                    for jb in range(NT):
                        nc.tensor.matmul(
                            pav[:],
                            eT[:, jb * P : (jb + 1) * P],
                            hp16[jb][:, h * (F + 1) : (h + 1) * (F + 1)],
                            start=(jb == 0),
                            stop=(jb == NT - 1),
                        )
                    rden = small.tile([P, 1], F32, tag="rden")
                    nc.vector.reciprocal(rden[:], pav[:, F : F + 1])

                    # fp32 attn output (paired 1MB DMAs)
                    if t % 2 == 0:
                        a32pair = work.tile([P, 2 * N], F32, tag="a32")
                    attn32 = a32pair[:, (t % 2) * N : (t % 2 + 1) * N]
                    nc.vector.tensor_scalar(
                        out=attn32,
                        in0=e[:],
                        scalar1=rden[:],
                        scalar2=2.0 ** -7,
                        op0=mybir.AluOpType.mult,
                        op1=mybir.AluOpType.mult,
                    )
                    if t % 2 == 1:
                        nc.sync.dma_start(
                            out=attn_d[
                                h, (t - 1) * P : (t + 1) * P, :
                            ].rearrange("(b p) j -> p b j", p=P),
                            in_=a32pair[:].rearrange("p (b j) -> p b j", j=N),
                        )

                    nc.vector.tensor_scalar_mul(
                        out_h[:, t * F : (t + 1) * F], pav[:, 0:F], rden[:]
                    )
                nc.sync.dma_start(
                    out=out_d[h].rearrange("(t p) f -> p t f", p=P),
                    in_=out_h[:].rearrange("p (t f) -> p t f", f=F),
                )
    nc.compile()
    return nc


def _get_nc():
    if "nc" not in _CACHE:
        _CACHE["nc"] = _build()
    return _CACHE["nc"]


def kernel(h, Relation, w, a_src, a_dst, bias_p):
    h = np.asarray(h, dtype=np.float32)
    Relation = np.ascontiguousarray(np.asarray(Relation, dtype=np.float32))
    w = np.asarray(w, dtype=np.float32)
    a_src = np.asarray(a_src, dtype=np.float32)
    a_dst = np.asarray(a_dst, dtype=np.float32)
    bias_p = np.asarray(bias_p, dtype=np.float32)

    hT = np.ascontiguousarray(h.transpose(0, 2, 1))  # [BS, F, N]
    ws = np.einsum("hfo,ho->hf", w, a_src[..., 0])  # [H, F]
    wd = np.einsum("hfo,ho->hf", w, a_dst[..., 0])
    sdw = np.ascontiguousarray(
        np.concatenate([ws.T, wd.T], axis=1), dtype=np.float32
    )  # [F, 16]
    w_pack = np.ascontiguousarray(
        w.transpose(1, 0, 2).reshape(F, H * F), dtype=np.float32
    )
    ones = np.ones((1, N), dtype=np.float32)

    nc = _get_nc()
    in_maps = [
        {"hT": hT[b], "R": Relation[b], "w": w_pack, "sdw": sdw, "ones": ones}
        for b in range(BS)
    ]
    res = run_bass_kernel_spmd(nc, in_maps, core_ids=list(range(BS)))

    output = np.stack([res.results[b]["out"] for b in range(BS)])  # [BS,H,N,F]
    attn = np.stack([res.results[b]["attn"] for b in range(BS)])  # [BS,H,N,N]

    # The device AV/denominator path uses e*2^-7 in fp16; rows whose max score
    # exceeds ~15.9 overflow to inf there (rden=0 -> zero row). Detect such
    # rows (row sum far from 1 / non-finite) and recompute them exactly on
    # host -- a handful of tail rows at most.
    rowsum = attn.sum(axis=-1)
    bad = ~np.isfinite(rowsum) | (np.abs(rowsum - 1.0) > 0.02)
    if bad.any():
        s_all = np.einsum("bnf,hf->bhn", h, ws)  # [BS,H,N]
        d_all = np.einsum("bnf,hf->bhn", h, wd)
        for b, hd in {(int(b_), int(h_)) for b_, h_, _ in np.argwhere(bad)}:
            rows = np.where(bad[b, hd])[0]
            hp = h[b] @ w[hd]  # [N, F]
            x = s_all[b, hd, rows][:, None] + d_all[b, hd][None, :]
            mm = Relation[b, rows, :] * np.where(x >= 0, x, 0.2 * x)
            ee = np.exp(mm - mm.max(axis=1, keepdims=True))
            arow = ee / ee.sum(axis=1, keepdims=True)
            attn[b, hd, rows, :] = arow
            output[b, hd, rows, :] = arow @ hp

    output = output + bias_p[None, None, None, :]
    return output, attn


# revision 9
# speedup vs baseline: 1.4801x; 1.0506x over previous
"""BatchMultiHeadGraphAttention Trainium2 kernel.

Data-parallel over batch: 8 batches -> 8 NeuronCores, one batch per core.

Per core (batch b), head h outer, i-tile t (128 rows) inner:
  scores   x[i,j] = s_i + d_j     PE: K=2 outer-product matmuls in float32r,
                                  hi+lo residual compensation -> exact fp32
                                  (fp32 matmuls are ~10x slower on HW; f32r
                                  rounds to ~tf32, the lo pass restores fp32)
           where s = h @ (w[h] @ a_src[h]), d = h @ (w[h] @ a_dst[h])
           (folded on host into sdw [64,16]; s/d rows built on-device)
  m        = leaky_relu(x * R)    DVE custom op MUL_LRELU (== R*leaky(x), R>=0)
  e        = exp(m)               ACT (no accum_out -- it costs ~4us/call on HW)
  e16      = e * 2^-7 in fp16     DVE (range-safe unnormalized; no recip dep)
  eT       = transpose(e16)       PE transpose, 8 blocks into one fp16 psum
                                  bank; one batched psum->sbuf copy (ACT)
  av|den   = eT.T @ [hp16 | 1]    PE fp16 matmuls, 65-wide rhs: the ones
                                  column makes the softmax denominator fall
                                  out of the AV accumulation for free
  rden     = 1/psum[:,64]         DVE (= 2^7/den)
  attn32   = e * rden * 2^-7      DVE -> DRAM attn (paired 1MB DMAs)
  out      = psum[:,0:64] * rden  DVE -> per-head buffer, one DMA per head
bias_p is added on host (zeros in the reference setup, but handled anyway).
GpSimd is avoided for streaming ops (~15x slower than DVE on HW).
"""

import os
import sys

import numpy as np

for _p in ("/opt/trn_rl_repo", "/root/.axon_site/_ro/trn_rl_repo"):
    if os.path.isdir(_p) and _p not in sys.path:
        sys.path.insert(0, _p)

import concourse.bass as bass  # noqa: E402
import concourse.tile as tile  # noqa: E402
from concourse import bacc, mybir  # noqa: E402
from concourse.bass_utils import run_bass_kernel_spmd  # noqa: E402
from concourse.masks import make_identity  # noqa: E402
from concourse import dve_ops as _dve_ops  # noqa: E402
from concourse.dve_spec import Spec, Src0, Src1, C0, maxx, lower as _dve_lower  # noqa: E402
from concourse.dve_uop import DveOpSpec  # noqa: E402

BS, N, H, F = 8, 1024, 8, 64
P = 128
NT = N // P  # 8 row tiles of 128
F32 = mybir.dt.float32
F32R = mybir.dt.float32r
F16 = mybir.dt.float16

_CACHE = {}


def _register_mul_lrelu():
    """Custom DVE op: out = leaky_relu(in0 * in1, s0) = max(u, s0*u), u = in0*in1.

    Fuses the Relation-mask multiply and the leaky relu into one DVE pass.
    """
    name = "MUL_LRELU_ANT"
    for op in _dve_ops.OPS:
        if op.name == name:
            return op
    u = Src0 * Src1
    spec = Spec(
        body=maxx(u, u * C0),
        reference=lambda in0, in1, s0, s1, imm2: np.maximum(
            (in0 * in1), s0 * (in0 * in1)
        ).astype(np.float32),
    )
    op = _dve_ops.DveOp(name, spec, subdim=False, uops_sha={})
    _dve_ops.OPS.append(op)
    _dve_ops._SUB_OPCODE_FOR_NAME[name] = (
        max(_dve_ops._SUB_OPCODE_FOR_NAME.values()) + 1
    )
    _dve_ops.CUSTOM_DVE_SPECS[name] = spec
    for ver in ("v3", "v4"):
        compiled = DveOpSpec(
            name=name,
            opcode=_dve_ops.get_dve_sub_opcode(name),
            uops=_dve_lower(spec, ver=ver),
            rd1_en=True,
        )
        op.uops_sha[ver] = compiled.sha(ver)
    return op


MUL_LRELU = _register_mul_lrelu()


def _build():
    nc = bacc.Bacc("TRN2", target_bir_lowering=False, debug=False)
    hT_d = nc.dram_tensor("hT", [F, N], F32, kind="ExternalInput").ap()
    R_d = nc.dram_tensor("R", [N, N], F32, kind="ExternalInput").ap()
    w_d = nc.dram_tensor("w", [F, H * F], F32, kind="ExternalInput").ap()
    sdw_d = nc.dram_tensor("sdw", [F, 2 * H], F32, kind="ExternalInput").ap()
    ones_d = nc.dram_tensor("ones", [1, N], F32R, kind="ExternalInput").ap()
    attn_d = nc.dram_tensor("attn", [H, N, N], F32, kind="ExternalOutput").ap()
    out_d = nc.dram_tensor("out", [H, N, F], F32, kind="ExternalOutput").ap()

    wb = int(os.environ.get("KB_WORK", "3"))
    with tile.TileContext(nc) as tc:
        with (
            tc.tile_pool(name="singles", bufs=1) as singles,
            tc.tile_pool(name="work", bufs=wb) as work,
            tc.tile_pool(name="stage", bufs=2) as stage,
            tc.tile_pool(name="small", bufs=4) as small,
            tc.tile_pool(name="ps_x", bufs=int(os.environ.get("KB_PSX", "2")), space="PSUM") as ps_x,
            tc.tile_pool(name="ps_t", bufs=int(os.environ.get("KB_PST", "2")), space="PSUM") as ps_t,
            tc.tile_pool(name="ps_av", bufs=int(os.environ.get("KB_PSAV", "2")), space="PSUM") as ps_av,
        ):
            # ---------- setup ----------
            hT_sb = singles.tile([F, N], F32, tag="hT")
            w_sb = singles.tile([F, H * F], F32, tag="w")
            sdw_sb = singles.tile([F, 2 * H], F32, tag="sdw")
            ones_r = singles.tile([1, N], F32R, tag="ones")
            nc.sync.dma_start(out=hT_sb, in_=hT_d)
            nc.sync.dma_start(out=w_sb, in_=w_d)
            nc.sync.dma_start(out=sdw_sb, in_=sdw_d)
            nc.sync.dma_start(out=ones_r, in_=ones_d)

            ident = singles.tile([P, P], F16, tag="ident")
            make_identity(nc, ident)

            # s/d row vectors for all heads: [16, N]; rows 0-7 = s_h, 8-15 = d_h
            ps_sd = ps_x.tile([16, N], F32, tag="x")
            for half in range(2):
                sl = slice(half * 512, (half + 1) * 512)
                nc.tensor.matmul(
                    ps_sd[:, sl], sdw_sb[:], hT_sb[:, sl], start=True, stop=True
                )
            sd_sb = singles.tile([16, N], F32, tag="sd")
            nc.vector.tensor_copy(sd_sb[:], ps_sd[:])
            # f32r rounding + residual for exact-score compensation
            sd_hi = singles.tile([16, N], F32R, tag="sdhi")
            nc.vector.tensor_copy(sd_hi[:], sd_sb[:])
            sd_lo32 = singles.tile([16, N], F32, tag="sdlo32")
            nc.vector.tensor_tensor(
                out=sd_lo32[:],
                in0=sd_sb[:],
                in1=sd_hi[:].bitcast(F32),
                op=mybir.AluOpType.subtract,
            )
            sd_lo = singles.tile([16, N], F32R, tag="sdlo")
            nc.vector.tensor_copy(sd_lo[:], sd_lo32[:])

            # h_prime in fp16 with an interleaved ones column per head:
            # hp16[nt][:, 65h:65h+64] = (h @ w[h])[j-tile], col 65h+64 = 1.0
            hp16 = []
            for nt in range(NT):
                ps_hp = ps_x.tile([P, H * F], F32, tag="x")
                for h in range(H):
                    nc.tensor.matmul(
                        ps_hp[:, h * F : (h + 1) * F],
                        hT_sb[:, nt * P : (nt + 1) * P],
                        w_sb[:, h * F : (h + 1) * F],
                        start=(h == 0),
                        stop=(h == H - 1),
                    )
                t16 = singles.tile([P, H * (F + 1)], F16, tag=f"hp{nt}")
                nc.vector.tensor_copy(
                    t16[:].rearrange("p (h f) -> p h f", f=F + 1)[:, :, 0:F],
                    ps_hp[:].rearrange("p (h f) -> p h f", f=F),
                )
                nc.vector.memset(
                    t16[:].rearrange("p (h f) -> p h f", f=F + 1)[:, :, F : F + 1],
                    1.0,
                )
                hp16.append(t16)

            # all Relation row-tiles resident (8 x 4KB/partition)
            R_all = []
            for t in range(NT):
                R_t = singles.tile([P, N], F32, tag=f"R{t}")
                nc.sync.dma_start(out=R_t, in_=R_d[t * P : (t + 1) * P, :])
                R_all.append(R_t)

            # ---------- main loop (h outer so score operands stage once/head) --
            for h in range(H):
                # score operands for head h (f32r), hi and lo:
                #   l_* [2, N]: row0 = s-part, row1 = ones   (lhsT source)
                #   r_* [2, N]: row0 = ones, row1 = d-part   (rhs source)
                stg = {}
                for kind, src_row, data_row in (
                    ("lhi", sd_hi[h : h + 1, :], 0),
                    ("rhi", sd_hi[H + h : H + h + 1, :], 1),
                    ("llo", sd_lo[h : h + 1, :], 0),
                    ("rlo", sd_lo[H + h : H + h + 1, :], 1),
                ):
                    st = stage.tile([2, N], F32R, tag=kind)
                    nc.sync.dma_start(
                        out=st[data_row : data_row + 1, :], in_=src_row
                    )
                    nc.sync.dma_start(
                        out=st[1 - data_row : 2 - data_row, :], in_=ones_r[:]
                    )
                    stg[kind] = st

                out_h = work.tile([P, NT * F], F32, tag="oh")
                for t in range(NT):
                    R_t = R_all[t]
                    tsl = slice(t * P, (t + 1) * P)

                    px = ps_x.tile([P, N], F32, tag="x")
                    for half in range(2):
                        sl = slice(half * 512, (half + 1) * 512)
                        nc.tensor.matmul(
                            px[:, sl], stg["lhi"][:, tsl], stg["rhi"][:, sl],
                            start=True, stop=False,
                        )
                        nc.tensor.matmul(
                            px[:, sl], stg["llo"][:, tsl], stg["rlo"][:, sl],
                            start=False, stop=True,
                        )

                    m = work.tile([P, N], F32, tag="m")
                    nc.vector._custom_dve(
                        MUL_LRELU, out=m[:], in0=px[:], in1=R_t[:], s0=0.2
                    )
                    e = work.tile([P, N], F32, tag="e")
                    nc.scalar.activation(
                        e[:], m[:], mybir.ActivationFunctionType.Exp
                    )
                    # fp16 copy of e scaled by 2^-7 (range-safe, no recip dep)
                    attn16 = work.tile([P, N], F16, tag="a16")
                    nc.vector.tensor_scalar_mul(attn16[:], e[:], 2.0 ** -7)

                    # 8 transposed blocks into ONE fp16 psum bank
                    pt = ps_t.tile([P, N], F16, tag="t")
                    for jb in range(NT):
                        nc.tensor.matmul(
                            pt[:, jb * P : (jb + 1) * P],
                            attn16[:, jb * P : (jb + 1) * P],
                            ident[:],
                            is_transpose=True,
                            start=(jb == 0),
                            stop=(jb == NT - 1),
                        )
                    eT = work.tile([P, N], F16, tag="eT")
                    if os.environ.get("KETC", "act") == "dve":
                        nc.vector.tensor_copy(eT[:], pt[:])
                    else:
                        nc.scalar.copy(eT[:], pt[:])

                    # AV + denominator: 65-wide rhs, ones column -> den*2^-7
                    pav = ps_av.tile([P, F + 1], F32, tag="av")
                    for jb in range(NT):
                        nc.tensor.matmul(
                            pav[:],
                            eT[:, jb * P : (jb + 1) * P],
                            hp16[jb][:, h * (F + 1) : (h + 1) * (F + 1)],
                            start=(jb == 0),
                            stop=(jb == NT - 1),
                        )
                    rden = small.tile([P, 1], F32, tag="rden")
                    nc.vector.reciprocal(rden[:], pav[:, F : F + 1])

                    # fp32 attn output (paired 1MB DMAs)
                    if t % 2 == 0:
                        a32pair = work.tile([P, 2 * N], F32, tag="a32")
                    attn32 = a32pair[:, (t % 2) * N : (t % 2 + 1) * N]
                    nc.vector.tensor_scalar(
                        out=attn32,
                        in0=e[:],
                        scalar1=rden[:],
                        scalar2=2.0 ** -7,
                        op0=mybir.AluOpType.mult,
                        op1=mybir.AluOpType.mult,
                    )
                    if t % 2 == 1:
                        nc.sync.dma_start(
                            out=attn_d[
                                h, (t - 1) * P : (t + 1) * P, :
                            ].rearrange("(b p) j -> p b j", p=P),
                            in_=a32pair[:].rearrange("p (b j) -> p b j", j=N),
                        )

                    nc.vector.tensor_scalar_mul(
                        out_h[:, t * F : (t + 1) * F], pav[:, 0:F], rden[:]
                    )
                nc.sync.dma_start(
                    out=out_d[h].rearrange("(t p) f -> p t f", p=P),
                    in_=out_h[:].rearrange("p (t f) -> p t f", f=F),
                )
    nc.compile()
    return nc


def _get_nc():
    if "nc" not in _CACHE:
        _CACHE["nc"] = _build()
    return _CACHE["nc"]


def kernel(h, Relation, w, a_src, a_dst, bias_p):
    h = np.asarray(h, dtype=np.float32)
    Relation = np.ascontiguousarray(np.asarray(Relation, dtype=np.float32))
    w = np.asarray(w, dtype=np.float32)
    a_src = np.asarray(a_src, dtype=np.float32)
    a_dst = np.asarray(a_dst, dtype=np.float32)
    bias_p = np.asarray(bias_p, dtype=np.float32)

    hT = np.ascontiguousarray(h.transpose(0, 2, 1))  # [BS, F, N]
    ws = np.einsum("hfo,ho->hf", w, a_src[..., 0])  # [H, F]
    wd = np.einsum("hfo,ho->hf", w, a_dst[..., 0])
    sdw = np.ascontiguousarray(
        np.concatenate([ws.T, wd.T], axis=1), dtype=np.float32
    )  # [F, 16]
    w_pack = np.ascontiguousarray(
        w.transpose(1, 0, 2).reshape(F, H * F), dtype=np.float32
    )
    ones = np.ones((1, N), dtype=np.float32)

    nc = _get_nc()
    in_maps = [
        {"hT": hT[b], "R": Relation[b], "w": w_pack, "sdw": sdw, "ones": ones}
        for b in range(BS)
    ]
    res = run_bass_kernel_spmd(nc, in_maps, core_ids=list(range(BS)))

    output = np.stack([res.results[b]["out"] for b in range(BS)])  # [BS,H,N,F]
    attn = np.stack([res.results[b]["attn"] for b in range(BS)])  # [BS,H,N,N]

    # The device AV/denominator path uses e*2^-7 in fp16; rows whose max score
    # exceeds ~15.9 overflow to inf there (rden=0 -> zero row). Detect such
    # rows (row sum far from 1 / non-finite) and recompute them exactly on
    # host -- a handful of tail rows at most.
    rowsum = attn.sum(axis=-1)
    bad = ~np.isfinite(rowsum) | (np.abs(rowsum - 1.0) > 0.02)
    if bad.any():
        s_all = np.einsum("bnf,hf->bhn", h, ws)  # [BS,H,N]
        d_all = np.einsum("bnf,hf->bhn", h, wd)
        for b, hd in {(int(b_), int(h_)) for b_, h_, _ in np.argwhere(bad)}:
            rows = np.where(bad[b, hd])[0]
            hp = h[b] @ w[hd]  # [N, F]
            x = s_all[b, hd, rows][:, None] + d_all[b, hd][None, :]
            mm = Relation[b, rows, :] * np.where(x >= 0, x, 0.2 * x)
            ee = np.exp(mm - mm.max(axis=1, keepdims=True))
            arow = ee / ee.sum(axis=1, keepdims=True)
            attn[b, hd, rows, :] = arow
            output[b, hd, rows, :] = arow @ hp

    output = output + bias_p[None, None, None, :]
    return output, attn
